# revision 5
# baseline (speedup 1.0000x reference)
"""BiAttention Trainium2 kernel (8 NeuronCores, batch-parallel).

Problem (per batch element b, 8 of them -> one per core):
    A_proj = A @ W_A + b_A            [2048, 64]
    B_proj = B @ W_B + b_B            [2048, 64]
    S      = A_proj @ B_proj^T        [2048, 2048]
    A_star = softmax(S, axis=-1) @ B  [2048, 768]
    B_star = softmax(S, axis=0)^T @ A [2048, 768]

Key algebra used on-device (|S| < ~30, so exp(S) is safe in f32/bf16
without max-subtraction):
    E = exp(S)
    A_star = diag(1/rowsum(E)) . (E @ B)
    B_star = diag(1/colsum(E)) . (E^T @ A)
rowsum/colsum come for free from a ones-column in the moving operand.

v2 design notes (all transposition moved off the TensorE):
  * input tiles are DMA'd (with f32->bf16 cast) straight into the `aug`
    moving-operand tensors; the d-major copies needed by the projections
    are produced by the DMA XBAR transpose (14ns per 16x128 tile) on the
    sync hwdge queue, not by identity matmuls on the PE.
  * only the dir-B score panels E[s, t-stripe] are computed on the PE
    (+ exp on ScalarE).  The dir-A panels E^T[t, s-stripe] are byte
    -identical transposes of the retained dir-B packs, produced by XBAR
    while earlier stripes accumulate.  This removes half the score
    matmuls and half the exp work.
  * projB is computed in just-in-time stripe chunks so the first dir-B
    work item only waits on A + the first quarter of B.
  * warmup/filler matmuls (no data deps) keep the PE clock gate at 8/8
    through the load phase.  They all precede the first real accum psum
    allocation so the psum ring slot they write is never live.
"""

import sys

if "/opt/trn_rl_repo" not in sys.path:
    sys.path.insert(0, "/opt/trn_rl_repo")

import numpy as np

import concourse.bass as bass
import concourse.mybir as mybir
import concourse.tile as tile
from concourse import bacc
from concourse.bass import ts
from concourse.bass_utils import run_bass_kernel_spmd

F32 = mybir.dt.float32
BF16 = mybir.dt.bfloat16
AF = mybir.ActivationFunctionType

L = 2048          # sequence length (both La and Lb)
D = 768           # model dim
H = 64            # projection dim
NT = L // 128     # 16 row/col tiles of 128
KD = D // 128     # 6 contraction tiles for the projections
NSUP = L // 512   # 4 supers (512-wide output stripes)
DP = D + 1        # moving operand width with the ones column
DPAD = 784        # aug row pitch: 784*2B = 1568, 32B-aligned for XBAR srcs

N_CORES = 8

_CACHE = {}


def _build():
    nc = bacc.Bacc("TRN2", target_bir_lowering=False, debug=False,
                   num_devices=N_CORES)
    A_d = nc.dram_tensor("A", [L, D], F32, kind="ExternalInput").ap()
    B_d = nc.dram_tensor("B", [L, D], F32, kind="ExternalInput").ap()
    WA_d = nc.dram_tensor("W_A", [D, H], F32, kind="ExternalInput").ap()
    WB_d = nc.dram_tensor("W_B", [D, H], F32, kind="ExternalInput").ap()
    bA_d = nc.dram_tensor("b_A", [H, 1], F32, kind="ExternalInput").ap()
    bB_d = nc.dram_tensor("b_B", [H, 1], F32, kind="ExternalInput").ap()
    AS_d = nc.dram_tensor("A_star", [L, D], F32, kind="ExternalOutput").ap()
    BS_d = nc.dram_tensor("B_star", [L, D], F32, kind="ExternalOutput").ap()

    with tile.TileContext(nc) as tc:
        with (
            tc.tile_pool(name="mov", bufs=1) as pmov,
            tc.tile_pool(name="mtsu", bufs=4) as pmts,
            tc.tile_pool(name="pack", bufs=34) as ppack,
            tc.tile_pool(name="pa", bufs=2) as ppa,
            tc.tile_pool(name="outp", bufs=4) as pout,
            tc.tile_pool(name="psum", bufs=2, space="PSUM") as pps,
        ):
            dram = {"A": A_d, "B": B_d}
            aug = {}
            projT = {}
            for side in ("A", "B"):
                # moving operand: cols 0:768 filled by the casting loads,
                # col 768 = ones (row pitch padded to 784 for XBAR align)
                aug[side] = pmov.tile([128, NT, DPAD], BF16, tag=f"aug{side}",
                                      name=f"{side}_aug")
                # rows 0:64 written by proj activation; rows 64:128 dup'd
                # so K=64 score matmuls can row-pack with tile_position
                projT[side] = pmov.tile([128, L], BF16, tag=f"p{side}",
                                        name=f"{side}_projT")

            w_sb = {}
            b_sb = {}

            def load_weights():
                for side, (W_dram, b_dram) in (
                    ("A", (WA_d, bA_d)), ("B", (WB_d, bB_d))
                ):
                    wb = pmov.tile([128, KD, H], BF16, tag=f"w{side}",
                                   name=f"w{side}b")
                    nc.gpsimd.dma_start(
                        out=wb, in_=W_dram.rearrange("(k p) h -> p k h", p=128)
                    )
                    bt = pmov.tile([H, 1], F32, tag=f"b{side}",
                                   name=f"b{side}sb")
                    nc.scalar.dma_start(out=bt, in_=b_dram)
                    w_sb[side] = wb
                    b_sb[side] = bt

            def load_unit(side, u, split=False):
                # casting DMA f32 DRAM -> bf16 straight into aug (SWDGE)
                if split:
                    for t in range(2):
                        i = 2 * u + t
                        nc.gpsimd.dma_start(out=aug[side][:, i, 0:D],
                                            in_=dram[side][ts(i, 128), :])
                else:
                    nc.gpsimd.dma_start(
                        out=aug[side][:, 2 * u:2 * u + 2, 0:D],
                        in_=dram[side][u * 256:(u + 1) * 256, :].rearrange(
                            "(t p) d -> p t d", p=128
                        ),
                    )

            def proj_tile(side, i):
                # mtsU = X^T blocks for s-tile i via XBAR (sync hwdge queue)
                mtsu = pmts.tile([128, KD, 128], BF16, tag="mtsu",
                                 name=f"mts{side}{i}")
                nc.sync.dma_start(out=mtsu, in_=aug[side][:, i, 0:D],
                                  transpose=True)
                # projT[h, s-tile i] = sum_d W[d,h] X^T[d,s]
                ps = pps.tile([128, 1024], F32, tag="spack",
                              name=f"psproj{side}{i}")
                for k in range(KD):
                    nc.tensor.matmul(
                        ps[0:H, 0:128],
                        w_sb[side][:, k, :],
                        mtsu[:, k, :],
                        start=(k == 0), stop=(k == KD - 1),
                    )
                nc.scalar.activation(
                    out=projT[side][0:H, ts(i, 128)], in_=ps[0:H, 0:128],
                    func=AF.Identity, bias=b_sb[side], scale=1.0,
                )

            def dup_proj(side, u):
                # duplicate stripe u into partitions 64:128 for row-packing
                nc.sync.dma_start(out=projT[side][H:128, ts(u, 512)],
                                  in_=projT[side][0:H, ts(u, 512)])

            def projB_chunk(u):
                for i in range(4 * u, 4 * u + 4):
                    proj_tile("B", i)
                dup_proj("B", u)

            # ---- prelude: loads + projections, PE idle covered by fillers
            warm = pmov.tile([128, 512], BF16, tag="warm", name="warm")
            nc.vector.memset(warm, 0.0)
            for side in ("A", "B"):
                nc.vector.memset(aug[side][:, :, D:DP], 1.0)

            load_weights()
            # A side first: dir-B work items need all of projT_A
            load_unit("A", 0, split=True)
            load_unit("A", 1, split=True)
            for u in range(2, 8):
                load_unit("A", u)
            for u in range(8):
                load_unit("B", u)

            # HAM warmup + fillers: dummy matmuls with no data deps so the
            # PE clock ramps to 8/8 and stays there through the load phase.
            wps = pps.tile([128, 1024], F32, tag="accum", name="warmps")

            def filler(n, width=512):
                for _ in range(n):
                    nc.tensor.matmul(wps[:, 0:width], warm[:, 0:128],
                                     warm[:, 0:width], start=True, stop=True)

            filler(16, 128)
            for i in range(NT):
                proj_tile("A", i)
                filler(4)
            for u in range(NSUP):
                dup_proj("A", u)
            projB_chunk(0)
            filler(6)

            # ---- main loop ----
            # dir "B" stripes: packs = E[s, t-stripe u] via PE scores + exp
            # dir "A" stripes: packs = E^T[t, s-stripe v] via XBAR of the
            #                  retained dir-B packs
            pkts = {}    # (u, jp) -> pack tile [128, 1024]
            PA = {}      # v -> [128, NT, 4, 128] stationary for dir A

            def emit_pack_piece(u, jps):
                for jp in jps:
                    pkt = ppack.tile([128, 1024], BF16, tag="pack", bufs=34,
                                     name=f"pkB{u}{jp}")
                    ps = pps.tile([128, 1024], F32, tag="spack",
                                  name=f"pssB{u}{jp}")
                    for h2 in range(2):
                        j = jp * 2 + h2
                        base = h2 * 64
                        nc.tensor.matmul(
                            ps[:, ts(h2, 512)],
                            projT["A"][base:base + H, ts(j, 128)],
                            projT["B"][base:base + H, ts(u, 512)],
                            start=True, stop=True,
                            tile_position=(base, 0),
                        )
                    nc.scalar.activation(out=pkt, in_=ps, func=AF.Exp)
                    pkts[(u, jp)] = pkt

            def emit_xbar(v):
                # PA[v][:, j, si, :] = E^T[t-tile j, s-tile 4v+si]
                pa_t = ppa.tile([128, NT, 4, 128], BF16, tag="pa",
                                name=f"paA{v}")
                for up in range(NSUP):
                    for jp in (2 * v, 2 * v + 1):
                        for h2 in range(2):
                            si = 2 * (jp - 2 * v) + h2
                            nc.sync.dma_start(
                                out=pa_t[:, 4 * up:4 * up + 4, si, :],
                                in_=pkts[(up, jp)][:, ts(h2, 512)],
                                transpose=True,
                            )
                PA[v] = pa_t

            def accum_block(dirn, u, ii, mv, out_d):
                pa = pps.tile([128, 1024], F32, tag="accum",
                              name=f"pac{dirn}{u}{ii}")
                for j in range(NT):
                    if dirn == "B":
                        lhs = pkts[(u, j // 2)][:, (j % 2) * 512 + ii * 128:
                                                (j % 2) * 512 + ii * 128 + 128]
                    else:
                        lhs = PA[u][:, j, ii, :]
                    # short mm first: the trailing 512-col mm covers the
                    # next tile's LDWEIGHTS pull-ahead window
                    nc.tensor.matmul(
                        pa[:, 512:DP], lhs, mv[:, j, 512:DP],
                        start=(j == 0), stop=(j == NT - 1),
                    )
                    nc.tensor.matmul(
                        pa[:, 0:512], lhs, mv[:, j, 0:512],
                        start=(j == 0), stop=(j == NT - 1),
                    )
                rinv = pout.tile([128, 1], F32, tag="rinv",
                                 name=f"ri{dirn}{u}{ii}")
                nc.vector.reciprocal(out=rinv, in_=pa[:, D:DP])
                ot = pout.tile([128, D], F32, tag="ot",
                               name=f"ot{dirn}{u}{ii}")
                nc.vector.tensor_scalar_mul(ot, pa[:, 0:D], rinv)
                nc.sync.dma_start(out=out_d[ts(u * 4 + ii, 128), :], in_=ot)

            def emit_item(dirn, u, after):
                # after: {ii: callable} emitted right after accum block ii so
                # their deps have time to land before the Tensor queue
                # reaches the emitted matmuls
                mv = aug["A"] if dirn == "B" else aug["B"]
                out_d = BS_d if dirn == "B" else AS_d
                for ii in range(4):
                    accum_block(dirn, u, ii, mv, out_d)
                    if after and ii in after:
                        after[ii]()

            emit_pack_piece(0, range(8))
            for u in range(3):
                nxt = u + 1
                emit_item("B", u, {
                    1: (lambda n=nxt: (projB_chunk(n),
                                       emit_pack_piece(n, range(4)))),
                    2: (lambda n=nxt: emit_pack_piece(n, range(4, 8))),
                })
            emit_item("B", 3, {0: lambda: emit_xbar(0)})
            emit_item("A", 0, {0: lambda: emit_xbar(1)})
            emit_item("A", 1, {0: lambda: emit_xbar(2)})
            emit_item("A", 2, {0: lambda: emit_xbar(3)})
            emit_item("A", 3, None)

    nc.compile()
    return nc


def _get_nc():
    if "nc" not in _CACHE:
        _CACHE["nc"] = _build()
    return _CACHE["nc"]


def _run(inputs, trace=False):
    nc = _get_nc()
    A = np.ascontiguousarray(np.asarray(inputs["A"], dtype=np.float32))
    B = np.ascontiguousarray(np.asarray(inputs["B"], dtype=np.float32))
    W_A = np.ascontiguousarray(np.asarray(inputs["W_A"], dtype=np.float32))
    W_B = np.ascontiguousarray(np.asarray(inputs["W_B"], dtype=np.float32))
    b_A = np.asarray(inputs["b_A"], dtype=np.float32).reshape(H, 1)
    b_B = np.asarray(inputs["b_B"], dtype=np.float32).reshape(H, 1)
    in_maps = [
        {
            "A": A[c], "B": B[c],
            "W_A": W_A, "W_B": W_B,
            "b_A": b_A, "b_B": b_B,
        }
        for c in range(N_CORES)
    ]
    res = run_bass_kernel_spmd(nc, in_maps, list(range(N_CORES)), trace=trace)
    A_star = np.stack([res.results[c]["A_star"] for c in range(N_CORES)])
    B_star = np.stack([res.results[c]["B_star"] for c in range(N_CORES)])
    return A_star, B_star, res


def kernel(**inputs):
    A_star, B_star, _ = _run(inputs)
    return A_star, B_star


# revision 6
# speedup vs baseline: 1.4016x; 1.4016x over previous
"""BiAttention Trainium2 kernel (8 NeuronCores, batch-parallel).

Problem (per batch element b, 8 of them -> one per core):
    A_proj = A @ W_A + b_A            [2048, 64]
    B_proj = B @ W_B + b_B            [2048, 64]
    S      = A_proj @ B_proj^T        [2048, 2048]
    A_star = softmax(S, axis=-1) @ B  [2048, 768]
    B_star = softmax(S, axis=0)^T @ A [2048, 768]

Key algebra used on-device (|S| < ~30, so exp(S) is safe in f32/bf16
without max-subtraction):
    E = exp(S)
    A_star = diag(1/rowsum(E)) . (E @ B)
    B_star = diag(1/colsum(E)) . (E^T @ A)
rowsum/colsum come for free from a ones-column in the moving operand.

E is never materialized in full: score panels are recomputed per
512-wide output stripe (K=64 contraction - cheap) directly from the
projections, exp'd into bf16 packs, and immediately consumed as the
stationary operand of the big matmuls.

v3 schedule notes:
  * casting loads write straight into the `aug` moving-operand tensors
    (no staging buffer / no vector copies); the PE transposes read aug
    slices as their stationary operand.
  * warmup/filler matmuls source a memset tile (no DMA dependency) so
    the PE clock gate ramps at ~7us and stays at 8/8 through the load
    phase.  All fillers precede the first real accum psum allocation.
  * projections are emitted in 512-col half-chunks and the late input
    units' transposes + projections + score packs are hooked between
    accum blocks of earlier work items, so the first accumulation
    starts as soon as B + the first quarter of A has landed.
"""

import sys

if "/opt/trn_rl_repo" not in sys.path:
    sys.path.insert(0, "/opt/trn_rl_repo")

import numpy as np
import ml_dtypes

import concourse.bass as bass
import concourse.mybir as mybir
import concourse.tile as tile
from concourse import bacc
from concourse.bass import ts
from concourse.bass_utils import run_bass_kernel_spmd

F32 = mybir.dt.float32
BF16 = mybir.dt.bfloat16
AF = mybir.ActivationFunctionType

L = 2048          # sequence length (both La and Lb)
D = 768           # model dim
H = 64            # projection dim
NT = L // 128     # 16 row/col tiles of 128
KD = D // 128     # 6 contraction tiles for the projections
NSUP = L // 512   # 4 supers (512-wide output stripes)
DP = D + 1        # moving operand width with the ones column

N_CORES = 8

_CACHE = {}

_IDENT = np.eye(128, dtype=ml_dtypes.bfloat16)


def _build():
    nc = bacc.Bacc("TRN2", target_bir_lowering=False, debug=False,
                   num_devices=N_CORES)
    A_d = nc.dram_tensor("A", [L, D], F32, kind="ExternalInput").ap()
    B_d = nc.dram_tensor("B", [L, D], F32, kind="ExternalInput").ap()
    WA_d = nc.dram_tensor("W_A", [D, H], F32, kind="ExternalInput").ap()
    WB_d = nc.dram_tensor("W_B", [D, H], F32, kind="ExternalInput").ap()
    bA_d = nc.dram_tensor("b_A", [H, 1], F32, kind="ExternalInput").ap()
    bB_d = nc.dram_tensor("b_B", [H, 1], F32, kind="ExternalInput").ap()
    ID_d = nc.dram_tensor("IDENT", [128, 128], BF16, kind="ExternalInput").ap()
    AS_d = nc.dram_tensor("A_star", [L, D], F32, kind="ExternalOutput").ap()
    BS_d = nc.dram_tensor("B_star", [L, D], F32, kind="ExternalOutput").ap()

    with tile.TileContext(nc) as tc:
        with (
            tc.tile_pool(name="mov", bufs=1) as pmov,
            tc.tile_pool(name="pack", bufs=18) as ppack,
            tc.tile_pool(name="outp", bufs=4) as pout,
            tc.tile_pool(name="psum", bufs=2, space="PSUM") as pps,
        ):
            ident = pmov.tile([128, 128], BF16, tag="ident", name="ident")
            warm = pmov.tile([128, 512], BF16, tag="warm", name="warm")

            dram = {"A": A_d, "B": B_d}
            aug = {}
            projT = {}
            mts = {}
            for side in ("A", "B"):
                # moving operand: cols 0:768 filled directly by the casting
                # loads, col 768 = ones via memset
                aug[side] = pmov.tile([128, NT, DP], BF16, tag=f"aug{side}",
                                      name=f"{side}_aug")
                # rows 0:64 written by proj activation; rows 64:128 dup'd so
                # K=64 score matmuls can row-pack with tile_position
                projT[side] = pmov.tile([128, L], BF16, tag=f"p{side}",
                                        name=f"{side}_projT")
                mts[side] = pmov.tile([128, NT * KD, 128], BF16,
                                      tag=f"t{side}", name=f"{side}_T")

            w_sb = {}
            b_sb = {}

            def load_weights():
                for side, (W_dram, b_dram) in (
                    ("B", (WB_d, bB_d)), ("A", (WA_d, bA_d))
                ):
                    wb = pmov.tile([128, KD, H], BF16, tag=f"w{side}",
                                   name=f"w{side}b")
                    nc.gpsimd.dma_start(
                        out=wb, in_=W_dram.rearrange("(k p) h -> p k h", p=128)
                    )
                    bt = pmov.tile([H, 1], F32, tag=f"b{side}",
                                   name=f"b{side}sb")
                    nc.scalar.dma_start(out=bt, in_=b_dram)
                    w_sb[side] = wb
                    b_sb[side] = bt

            def load_unit(side, u, split=False):
                # casting DMA f32 DRAM -> bf16 straight into aug (SWDGE)
                if split:
                    for t in range(2):
                        i = 2 * u + t
                        nc.gpsimd.dma_start(out=aug[side][:, i, 0:D],
                                            in_=dram[side][ts(i, 128), :])
                else:
                    nc.gpsimd.dma_start(
                        out=aug[side][:, 2 * u:2 * u + 2, 0:D],
                        in_=dram[side][u * 256:(u + 1) * 256, :].rearrange(
                            "(t p) d -> p t d", p=128
                        ),
                    )

            def trans_unit(side, u):
                # transpose the unit's 2x6 blocks on TensorE: psum <- block.T
                for t in range(2):
                    i = 2 * u + t
                    ps = pps.tile([128, 1024], F32, tag="spack",
                                  name=f"pstr{side}{i}")
                    for j in range(KD):
                        nc.tensor.matmul(ps[:, ts(j, 128)],
                                         aug[side][:, i, ts(j, 128)],
                                         ident, start=True, stop=True)
                    nc.scalar.copy(
                        out=mts[side][:, i * KD:(i + 1) * KD, :],
                        in_=ps[:, 0:KD * 128],
                    )

            def proj_hc(side, hc):
                # projT[h, s] over a 512-wide half-chunk (s-tiles 4hc..4hc+3)
                mtv = mts[side].rearrange("p (i j) q -> p i j q", j=KD)
                ps = pps.tile([128, 1024], F32, tag="spack",
                              name=f"psproj{side}{hc}")
                i0 = hc * 4
                for k in range(KD):
                    nc.tensor.matmul(
                        ps[:H, 0:512],
                        w_sb[side][:, k, :],
                        mtv[:, i0:i0 + 4, k, :],
                        start=(k == 0), stop=(k == KD - 1),
                    )
                nc.scalar.activation(
                    out=projT[side][0:H, ts(hc, 512)], in_=ps[:H, 0:512],
                    func=AF.Identity, bias=b_sb[side], scale=1.0,
                )
                # duplicate into partitions 64:128 for row-packed S matmuls
                nc.sync.dma_start(out=projT[side][H:128, ts(hc, 512)],
                                  in_=projT[side][0:H, ts(hc, 512)])

            # ---- prelude ----
            nc.vector.memset(warm, 0.0)
            for side in ("A", "B"):
                nc.vector.memset(aug[side][:, :, D:DP], 1.0)
            nc.sync.dma_start(out=ident, in_=ID_d)
            load_weights()
            load_unit("B", 0, split=True)
            load_unit("B", 1, split=True)
            for u in range(2, 8):
                load_unit("B", u)
            for u in range(8):
                load_unit("A", u)

            # HAM warmup + fillers: no data deps; keep the PE clock at 8/8
            # through the load phase.  All precede the first accum alloc so
            # their psum ring slot is never live.
            wps = pps.tile([128, 1024], F32, tag="accum", name="warmps")

            def filler(n, width=512):
                for _ in range(n):
                    nc.tensor.matmul(wps[:, 0:width], warm[:, 0:128],
                                     warm[:, 0:width], start=True, stop=True)

            filler(14, 128)
            for u in range(8):
                trans_unit("B", u)
                filler(2)
                if u % 2 == 1:
                    proj_hc("B", u // 2)
                    filler(2)
            for u in range(2):
                trans_unit("A", u)
                filler(2)
            proj_hc("A", 0)
            filler(2)

            # ---- main loop ----
            # dirn "A": A_star rows; panels E^T[t, s-stripe]
            #   (lhsT = B_projT tiles, rhs = A_projT stripe), moving = aug_B
            # dirn "B": B_star rows; panels E[s, t-stripe]
            #   (lhsT = A_projT tiles, rhs = B_projT stripe), moving = aug_A
            spec = {
                "A": (projT["B"], projT["A"], aug["B"], AS_d),
                "B": (projT["A"], projT["B"], aug["A"], BS_d),
            }
            pkts = {}

            def emit_pack_piece(dirn, u, jps):
                pT_l, pT_r, _, _ = spec[dirn]
                for jp in jps:
                    pkt = ppack.tile([128, 1024], BF16, tag="pack", bufs=18,
                                     name=f"pk{dirn}{u}{jp}")
                    ps = pps.tile([128, 1024], F32, tag="spack",
                                  name=f"pss{dirn}{u}{jp}")
                    for h2 in range(2):
                        j = jp * 2 + h2
                        base = h2 * 64
                        nc.tensor.matmul(
                            ps[:, ts(h2, 512)],
                            pT_l[base:base + H, ts(j, 128)],
                            pT_r[base:base + H, ts(u, 512)],
                            start=True, stop=True,
                            tile_position=(base, 0),
                        )
                    nc.scalar.activation(out=pkt, in_=ps, func=AF.Exp)
                    pkts[(dirn, u, jp)] = pkt

            def accum_block(dirn, u, ii):
                _, _, mv, out_d = spec[dirn]
                pa = pps.tile([128, 1024], F32, tag="accum",
                              name=f"pac{dirn}{u}{ii}")
                for j in range(NT):
                    lhs = pkts[(dirn, u, j // 2)][
                        :, (j % 2) * 512 + ii * 128:(j % 2) * 512 + ii * 128 + 128]
                    # short mm first: the trailing 512-col mm covers the
                    # next tile's LDWEIGHTS pull-ahead window
                    nc.tensor.matmul(
                        pa[:, 512:DP], lhs, mv[:, j, 512:DP],
                        start=(j == 0), stop=(j == NT - 1),
                    )
                    nc.tensor.matmul(
                        pa[:, 0:512], lhs, mv[:, j, 0:512],
                        start=(j == 0), stop=(j == NT - 1),
                    )
                rinv = pout.tile([128, 1], F32, tag="rinv",
                                 name=f"ri{dirn}{u}{ii}")
                nc.vector.reciprocal(out=rinv, in_=pa[:, D:DP])
                ot = pout.tile([128, D], F32, tag="ot",
                               name=f"ot{dirn}{u}{ii}")
                nc.vector.tensor_scalar_mul(ot, pa[:, 0:D], rinv)
                nc.sync.dma_start(out=out_d[ts(u * 4 + ii, 128), :], in_=ot)

            def emit_item(dirn, u, after):
                # after: {ii: [callables]} emitted right after accum block ii
                # so their deps have time to land before the Tensor queue
                # reaches the emitted matmuls
                for ii in range(4):
                    accum_block(dirn, u, ii)
                    if after and ii in after:
                        for fn in after[ii]:
                            fn()

            def prep_a(u2):
                # transposes + projection half-chunk for A units 2u2, 2u2+1
                def fn():
                    trans_unit("A", 2 * u2)
                    trans_unit("A", 2 * u2 + 1)
                    proj_hc("A", u2)
                return fn

            def pack_fn(dirn, u, jps):
                return lambda: emit_pack_piece(dirn, u, jps)

            emit_pack_piece("A", 0, range(8))
            emit_item("A", 0, {0: [prep_a(1)],
                               1: [pack_fn("A", 1, range(4))],
                               2: [pack_fn("A", 1, range(4, 8))]})
            emit_item("A", 1, {0: [prep_a(2)],
                               1: [pack_fn("A", 2, range(4))],
                               2: [pack_fn("A", 2, range(4, 8))]})
            emit_item("A", 2, {0: [prep_a(3)],
                               1: [pack_fn("A", 3, range(4))],
                               2: [pack_fn("A", 3, range(4, 8))]})
            emit_item("A", 3, {1: [pack_fn("B", 0, range(4))],
                               2: [pack_fn("B", 0, range(4, 8))]})
            emit_item("B", 0, {1: [pack_fn("B", 1, range(4))],
                               2: [pack_fn("B", 1, range(4, 8))]})
            emit_item("B", 1, {1: [pack_fn("B", 2, range(4))],
                               2: [pack_fn("B", 2, range(4, 8))]})
            emit_item("B", 2, {1: [pack_fn("B", 3, range(4))],
                               2: [pack_fn("B", 3, range(4, 8))]})
            emit_item("B", 3, None)

    nc.compile()
    return nc


def _get_nc():
    if "nc" not in _CACHE:
        _CACHE["nc"] = _build()
    return _CACHE["nc"]


def _run(inputs, trace=False):
    nc = _get_nc()
    A = np.ascontiguousarray(np.asarray(inputs["A"], dtype=np.float32))
    B = np.ascontiguousarray(np.asarray(inputs["B"], dtype=np.float32))
    W_A = np.ascontiguousarray(np.asarray(inputs["W_A"], dtype=np.float32))
    W_B = np.ascontiguousarray(np.asarray(inputs["W_B"], dtype=np.float32))
    b_A = np.asarray(inputs["b_A"], dtype=np.float32).reshape(H, 1)
    b_B = np.asarray(inputs["b_B"], dtype=np.float32).reshape(H, 1)
    in_maps = [
        {
            "A": A[c], "B": B[c],
            "W_A": W_A, "W_B": W_B,
            "b_A": b_A, "b_B": b_B,
            "IDENT": _IDENT,
        }
        for c in range(N_CORES)
    ]
    res = run_bass_kernel_spmd(nc, in_maps, list(range(N_CORES)), trace=trace)
    A_star = np.stack([res.results[c]["A_star"] for c in range(N_CORES)])
    B_star = np.stack([res.results[c]["B_star"] for c in range(N_CORES)])
    return A_star, B_star, res


def kernel(**inputs):
    A_star, B_star, _ = _run(inputs)
    return A_star, B_star
